# revision 1
# baseline (speedup 1.0000x reference)
"""AttentionBlock (GroupNorm32 + self/cross attention + proj + residual) on 8 TRN2 cores.

Sharding: data-parallel over batch. B=8 samples, one per NeuronCore. Each core runs
the full per-sample block: GroupNorm -> qkv/ekv projections -> 8-head attention
(encoder kv concat + additive mask) -> out projection -> residual.

Layout/scheduling notes (per core):
  - channel-major tensors are stored as [128, 4, *] with c = 128*i + p.
  - attention logits computed TRANSPOSED, wgtT[s, t]; the per-s mask becomes a
    per-partition bias; softmax denominators ride an appended ones-column in
    v^T (PV psum row 64).
  - softmax exp is SPLIT across engines per (head, s-tile): ScalarE ACT exp
    for head 0 (+ head 1 j<2), Schraudolph bit-trick exp on VectorE
    (tensor_scalar -> int16 write, bitcast-viewed bf16) for the rest, so the
    logit evacuation is not serialized on one engine.
  - phase pipeline per head pair p: the QK j-loop (self-pipelining on 3 PSUM
    slots) is interleaved with pair p+1's qkv/ekv/v-transpose units; then two
    DENSE per-head 18-matmul PV blocks keep the PE HAM clock-gate warm, with
    the leftover projection units as PE filler while head 0's normalize chain
    (denom copy -> approx reciprocal -> partition_broadcast -> fused
    normalize-multiply) drains.
  - PSUM plan (8 banks): tag "qk" [128,1024]f32 x3 bufs = 6 banks (shared by
    QK tiles and qkv/ekv/transpose/proj groups), tag "pv" [65,2,512]f32 x1 = 2.
  - q/k/v/ekv PSUM evacuation on ScalarE (Identity + per-partition bias).
  - partition_broadcast only writes correctly at output base partition 0, so
    each head gets its own [64, T] reciprocal tile.
"""

import sys
from contextlib import ExitStack

import numpy as np

for _p in ("/opt/trn_rl_repo",):
    if _p not in sys.path:
        sys.path.insert(0, _p)

import ml_dtypes  # noqa: E402

import concourse.bass as bass  # noqa: E402
import concourse.tile as tile  # noqa: E402
from concourse import bacc, mybir  # noqa: E402
from concourse.masks import make_identity  # noqa: E402

F32 = mybir.dt.float32
BF16 = mybir.dt.bfloat16
I16 = mybir.dt.int16
AF = mybir.ActivationFunctionType
ALU = mybir.AluOpType

B, C, HH, WW = 8, 512, 32, 32
T = HH * WW          # 1024
HEADS, CH, S = 8, 64, 77
ST = S + T           # 1101
NS = (ST + 127) // 128  # 9 s-tiles
GROUPS, GCH = 32, 16
N_CORES = 8

# Schraudolph bf16 exp: y ~= bitcast16(int16(A16*x + B16))
A16 = 128.0 / float(np.log(2.0))     # 184.6627
B16 = 127.0 * 128.0 - 6.35           # mid-point correction
MASK_NEG = -30.0                     # additive mask value (exp(-30) ~ 1e-13)

# v^T per head is [s, 65]: cols 0..63 = v channels, col 64 = ones (rowsum).
# vT2 tile is [128, NS, 2, VTC]; head hh at [..., hh, 0:65].
VTC = 66


def _exp_engine(hh: int, j: int) -> str:
    """Which engine computes exp for (head-in-pair hh, s-tile j)."""
    return "dve" if (hh == 1 and j >= 2) else "act"


DEBUG_DUMPS = False


def build_program():
    nc = bacc.Bacc("TRN2", target_bir_lowering=False, debug=False)

    x_d = nc.dram_tensor("x", [128, 4, T], F32, kind="ExternalInput")
    enc_d = nc.dram_tensor("enc", [128, 4, S], BF16, kind="ExternalInput")
    addma_d = nc.dram_tensor("addma", [128, 1], F32, kind="ExternalInput")
    addmd_d = nc.dram_tensor("addmd", [128, 1], F32, kind="ExternalInput")
    wq_d = nc.dram_tensor("wq", [128, 4, 3 * C], BF16, kind="ExternalInput")
    we_d = nc.dram_tensor("we", [128, 4, 2 * C], BF16, kind="ExternalInput")
    wp_d = nc.dram_tensor("wp", [128, 4, C], BF16, kind="ExternalInput")
    qb_d = nc.dram_tensor("qb", [128, 4, 3], F32, kind="ExternalInput")
    eb_d = nc.dram_tensor("eb", [128, 4, 2], F32, kind="ExternalInput")
    pb_d = nc.dram_tensor("pb", [128, 4], F32, kind="ExternalInput")
    gam_d = nc.dram_tensor("gam", [128, 4], F32, kind="ExternalInput")
    bet_d = nc.dram_tensor("bet", [128, 4], F32, kind="ExternalInput")
    out_d = nc.dram_tensor("out", [128, 4, T], F32, kind="ExternalOutput")
    if DEBUG_DUMPS:
        dbg_att_d = nc.dram_tensor("dbg_att", [128, 4, T], F32, kind="ExternalOutput")
        dbg_den_d = nc.dram_tensor("dbg_den", [4, 2, 2, 512], F32, kind="ExternalOutput")
        dbg_rec_d = nc.dram_tensor("dbg_rec", [4, 128, T], F32, kind="ExternalOutput")
        dbg_wgt_d = nc.dram_tensor("dbg_wgt", [128, 2, T], F32, kind="ExternalOutput")
        dbg_vt_d = nc.dram_tensor("dbg_vt", [128, NS, 2, VTC], F32, kind="ExternalOutput")

    with tile.TileContext(nc) as tc, ExitStack() as ctx:
        consts = ctx.enter_context(tc.tile_pool(name="consts", bufs=1))
        stats = ctx.enter_context(tc.tile_pool(name="stats", bufs=1))
        hp = ctx.enter_context(tc.tile_pool(name="hp", bufs=2))
        wgtp = ctx.enter_context(tc.tile_pool(name="wgtp", bufs=20))
        psum = ctx.enter_context(tc.tile_pool(name="psum", bufs=2, space="PSUM"))

        # ---- constant loads -------------------------------------------------
        x_sb = consts.tile([128, 4, T], F32)
        for i in range(4):
            nc.sync.dma_start(out=x_sb[:, i, :], in_=x_d.ap()[:, i, :])
        enc_sb = consts.tile([128, 4, S], BF16)
        nc.sync.dma_start(out=enc_sb, in_=enc_d.ap())
        wq = consts.tile([128, 4, 3 * C], BF16)
        nc.sync.dma_start(out=wq, in_=wq_d.ap())
        we = consts.tile([128, 4, 2 * C], BF16)
        nc.sync.dma_start(out=we, in_=we_d.ap())
        wp = consts.tile([128, 4, C], BF16)
        nc.sync.dma_start(out=wp, in_=wp_d.ap())
        addma = consts.tile([128, 1], F32)
        nc.sync.dma_start(out=addma, in_=addma_d.ap())
        addmd = consts.tile([128, 1], F32)
        nc.sync.dma_start(out=addmd, in_=addmd_d.ap())
        qb = consts.tile([128, 4, 3], F32)
        nc.sync.dma_start(out=qb, in_=qb_d.ap())
        eb = consts.tile([128, 4, 2], F32)
        nc.sync.dma_start(out=eb, in_=eb_d.ap())
        pb = consts.tile([128, 4], F32)
        nc.sync.dma_start(out=pb, in_=pb_d.ap())
        gam = consts.tile([128, 4], F32)
        nc.sync.dma_start(out=gam, in_=gam_d.ap())
        bet = consts.tile([128, 4], F32)
        nc.sync.dma_start(out=bet, in_=bet_d.ap())

        identf = consts.tile([128, 128], F32)
        make_identity(nc, identf)
        ident = consts.tile([128, 128], BF16)
        make_identity(nc, ident)

        # ---- GroupNorm(32) stats -------------------------------------------
        mv = stats.tile([128, 4, 2], F32)
        for i in range(4):
            bnst = stats.tile([128, 2, 6], F32, tag="bnst", bufs=2)
            nc.vector.bn_stats(out=bnst[:, 0, :], in_=x_sb[:, i, 0:512])
            nc.vector.bn_stats(out=bnst[:, 1, :], in_=x_sb[:, i, 512:1024])
            nc.vector.bn_aggr(out=mv[:, i, :], in_=bnst)

        stm = stats.tile([128, 4], F32)
        nc.vector.tensor_copy(out=stm, in_=mv[:, :, 0])
        stx = stats.tile([128, 4], F32)
        nc.vector.tensor_mul(out=stx, in0=mv[:, :, 0], in1=mv[:, :, 0])
        nc.vector.tensor_add(out=stx, in0=stx, in1=mv[:, :, 1])

        stmT = stats.tile([4, 128], F32)
        stxT = stats.tile([4, 128], F32)
        for src_t, dst_t in ((stm, stmT), (stx, stxT)):
            t_ps = psum.tile([4, 128], F32, tag="qk", bufs=3, name=f"tps_{src_t.tensor.name}")
            nc.tensor.transpose(t_ps, src_t, identf)
            nc.vector.tensor_copy(out=dst_t, in_=t_ps)

        gm = stats.tile([4, 8], F32)
        gx = stats.tile([4, 8], F32)
        nc.vector.reduce_sum(
            out=gm, in_=stmT.rearrange("p (g k) -> p g k", k=GCH),
            axis=mybir.AxisListType.X)
        nc.vector.reduce_sum(
            out=gx, in_=stxT.rearrange("p (g k) -> p g k", k=GCH),
            axis=mybir.AxisListType.X)
        mug = stats.tile([4, 8], F32)
        nc.vector.tensor_scalar_mul(out=mug, in0=gm, scalar1=1.0 / GCH)
        varg = stats.tile([4, 8], F32)
        nc.vector.tensor_mul(out=varg, in0=mug, in1=mug)
        nc.vector.tensor_scalar_mul(out=gx, in0=gx, scalar1=1.0 / GCH)
        nc.vector.tensor_sub(out=varg, in0=gx, in1=varg)
        eps_t = stats.tile([4, 1], F32)
        nc.vector.memset(eps_t, 1e-5)
        lnv = stats.tile([4, 8], F32)
        nc.scalar.activation(out=lnv, in_=varg, func=AF.Ln, bias=eps_t, scale=1.0)
        rstdg = stats.tile([4, 8], F32)
        nc.scalar.activation(out=rstdg, in_=lnv, func=AF.Exp, scale=-0.5)

        mu_col = stats.tile([128, 4], F32)
        rstd_col = stats.tile([128, 4], F32)
        for src_t, dst_t in ((mug, mu_col), (rstdg, rstd_col)):
            t_ps = psum.tile([8, 4], F32, tag="qk", bufs=3, name=f"tbps_{src_t.tensor.name}")
            nc.tensor.transpose(t_ps, src_t, identf[0:4, 0:4])
            t_sb = stats.tile([8, 4], F32, name=f"tsb_{src_t.tensor.name}")
            nc.vector.tensor_copy(out=t_sb, in_=t_ps)
            nc.sync.dma_start(
                out=dst_t,
                in_=bass.AP(
                    tensor=t_sb.tensor, offset=t_sb.offset,
                    ap=[list(t_sb.ap[0]), [0, GCH], list(t_sb.ap[-1])],
                ),
            )

        a_col = stats.tile([128, 4], F32)
        nc.vector.tensor_mul(out=a_col, in0=rstd_col, in1=gam)
        b_col = stats.tile([128, 4], F32)
        nc.vector.tensor_mul(out=b_col, in0=mu_col, in1=a_col)
        nc.vector.tensor_sub(out=b_col, in0=bet, in1=b_col)

        nrm = consts.tile([128, 4, T], BF16)
        for i in range(4):
            nc.vector.tensor_scalar(
                out=nrm[:, i, :], in0=x_sb[:, i, :],
                scalar1=a_col[:, i:i + 1], scalar2=b_col[:, i:i + 1],
                op0=ALU.mult, op1=ALU.add,
            )

        att_all = consts.tile([128, 4, T], BF16)

        # ---- attention: phase-pipelined over head pairs --------------------
        # Phase p: QK(p) j-loop + exp(p) (split ACT/DVE), interleaved with
        # pair p+1's qkv/ekv projections and DMA-xbar v-transposes; then a
        # dense 36-matmul PV(p) block (keeps HAM warm); then batched
        # normalize. qkv/ekv groups share the "pv" PSUM tag slots.
        qq = {}
        kk = {}
        vv = {}
        vT = {}
        wgts = {}

        def emit_qkv_unit(p, unit):
            """One projection unit for pair p: ('qkv',tci,bi) | ('ekv',bi) | ('vT',j)."""
            kind = unit[0]
            if kind == "qkv":
                _, tci, bi = unit
                tsl = slice(512 * tci, 512 * (tci + 1))
                dest = [qq[p][:, tsl],
                        kk[p][:, S + 512 * tci: S + 512 * (tci + 1)],
                        vv[p][:, S + 512 * tci: S + 512 * (tci + 1)]][bi]
                mm_ps = psum.tile([128, 512], F32, tag="qk", bufs=3,
                                  name=f"mm_{p}_{tci}_{bi}")
                for kc in range(4):
                    nc.tensor.matmul(
                        mm_ps,
                        wq[:, kc, 384 * p + 128 * bi: 384 * p + 128 * (bi + 1)],
                        nrm[:, kc, tsl],
                        start=(kc == 0), stop=(kc == 3),
                    )
                nc.scalar.activation(
                    out=dest, in_=mm_ps, func=AF.Identity,
                    bias=qb[:, p, bi:bi + 1], scale=1.0)
            elif kind == "ekv":
                _, bi = unit
                dest = [kk[p][:, 0:S], vv[p][:, 0:S]][bi]
                ek_ps = psum.tile([128, S], F32, tag="qk", bufs=3,
                                  name=f"ek_{p}_{bi}")
                for kc in range(4):
                    nc.tensor.matmul(
                        ek_ps,
                        we[:, kc, 256 * p + 128 * bi: 256 * p + 128 * (bi + 1)],
                        enc_sb[:, kc, :],
                        start=(kc == 0), stop=(kc == 3),
                    )
                nc.scalar.activation(
                    out=dest, in_=ek_ps, func=AF.Identity,
                    bias=eb[:, p, bi:bi + 1], scale=1.0)
            else:  # vT via PE transpose + one 3D DVE copy
                _, j = unit
                if j == 0:
                    nc.gpsimd.memset(vT[p][:, :, :, CH:CH + 1], 1.0)
                s0 = 128 * j
                rows = min(128, ST - s0)
                tr_ps = psum.tile([128, 128], BF16, tag="qk", bufs=3,
                                  name=f"tr_{p}_{j}")
                nc.tensor.transpose(
                    tr_ps[0:rows, :], vv[p][:, s0:s0 + rows], ident)
                nc.vector.tensor_copy(
                    out=vT[p][0:rows, j, :, 0:CH],
                    in_=tr_ps[0:rows, :].rearrange("p (h c) -> p h c", c=CH))

        def alloc_pair(p):
            qq[p] = hp.tile([128, T], BF16, tag="qq2", name=f"qq_{p}")
            kk[p] = hp.tile([128, ST], BF16, tag="kk2", name=f"kk_{p}")
            vv[p] = hp.tile([128, ST], BF16, tag="vv2", name=f"vv_{p}")
            vT[p] = hp.tile([128, NS, 2, VTC], BF16, tag="vT2", name=f"vT_{p}")

        def proj_units(p):
            return ([("qkv", tci, bi) for tci in range(2) for bi in range(3)]
                    + [("ekv", 0), ("ekv", 1)]
                    + [("vT", j) for j in range(NS)])

        def emit_qk_exp(p, j):
            s0 = 128 * j
            rows = min(128, ST - s0)
            ssl = slice(s0, s0 + rows)
            qk = [psum.tile([128, T], F32, tag="qk", bufs=3,
                            name=f"qk_{p}_{j}_{hh}") for hh in range(2)]
            for hh in range(2):
                rlo = 64 * hh
                for tci in range(2):
                    nc.tensor.matmul(
                        qk[hh][0:rows, 512 * tci:512 * (tci + 1)],
                        kk[p][rlo:rlo + 64, ssl],
                        qq[p][rlo:rlo + 64, 512 * tci:512 * (tci + 1)],
                    )
            for hh in range(2):
                w = wgtp.tile([128, T], BF16, tag="wgt", name=f"wgt_{p}_{j}_{hh}")
                wgts[(p, j, hh)] = w
                if _exp_engine(hh, j) == "act":
                    nc.scalar.activation(
                        out=w[0:rows, :], in_=qk[hh][0:rows, :],
                        func=AF.Exp, scale=0.125,
                        bias=(addma[0:rows] if j == 0 else 0.0),
                    )
                else:
                    nc.vector.tensor_scalar(
                        out=w.bitcast(I16)[0:rows, :],
                        in0=qk[hh][0:rows, :],
                        scalar1=A16 * 0.125,
                        scalar2=(addmd[0:rows] if j == 0 else B16),
                        op0=ALU.mult, op1=ALU.add,
                    )

        def emit_pv_hh(p, pv, j, hh):
            rows = min(128, ST - 128 * j)
            w = wgts[(p, j, hh)]
            for tci in range(2):
                nc.tensor.matmul(
                    pv[:, tci, :],
                    vT[p][0:rows, j, hh, 0:65],
                    w[0:rows, 512 * tci:512 * (tci + 1)],
                    start=(j == 0), stop=(j == NS - 1),
                    skip_group_check=True,
                )

        def emit_normalize_hh(p, pv, hh):
            # NOTE: partition_broadcast only writes correctly at output base
            # partition 0 -- use a separate [64, T] tile per head.
            den_h = hp.tile([1, 2, 512], F32, tag=f"den{hh}",
                            name=f"den_{p}_{hh}")
            nc.scalar.activation(
                out=den_h, in_=pv[64:65, :, :],
                func=AF.Copy, scale=1.0)
            rec_h = hp.tile([1, 2, 512], F32, tag=f"rec{hh}",
                            name=f"rec_{p}_{hh}")
            nc.vector.reciprocal_approx_fast(out=rec_h, in_=den_h)
            recipb_h = hp.tile([64, T], F32, tag=f"recipb{hh}",
                               name=f"recipb_{p}_{hh}")
            nc.gpsimd.partition_broadcast(recipb_h, rec_h)
            rlo = 64 * hh
            for tci in range(2):
                nc.vector.tensor_mul(
                    out=att_all[rlo:rlo + 64, p, 512 * tci:512 * (tci + 1)],
                    in0=pv[0:64, tci, :],
                    in1=recipb_h[:, 512 * tci:512 * (tci + 1)],
                )

        if DEBUG_DUMPS:
            dbg_wgt = consts.tile([128, 2, T], F32, name="dbg_wgt_sb")
            dbg_vt = consts.tile([128, NS, 2, VTC], F32, name="dbg_vt_sb")

        # prologue: pair 0 projections, emitted densely
        alloc_pair(0)
        for u in proj_units(0):
            emit_qkv_unit(0, u)

        for p in range(4):
            if p < 3:
                alloc_pair(p + 1)
                units = proj_units(p + 1)
            else:
                units = []
            # j-loop: one interleave unit per step (9 of ~17); the rest fill
            # the gap between the two dense per-head PV blocks below.
            for j in range(NS):
                emit_qk_exp(p, j)
                if units:
                    take = 1 if j < 8 else 2
                    for u in units[:take]:
                        emit_qkv_unit(p + 1, u)
                    units = units[take:]
            pv_ps = {}
            for hh in range(2):
                pv_ps[hh] = psum.tile([65, 2, 512], F32, tag="pv", bufs=1,
                                      name=f"pv_{p}_{hh}")
                for j in range(NS):
                    emit_pv_hh(p, pv_ps[hh], j, hh)
                emit_normalize_hh(p, pv_ps[hh], hh)
                if hh == 0:
                    # PE filler while head 0's normalize chain drains
                    for u in units:
                        emit_qkv_unit(p + 1, u)
                    units = []

        if DEBUG_DUMPS:
            att_f32 = consts.tile([128, 4, T], F32, name="att_f32")
            nc.vector.tensor_copy(out=att_f32, in_=att_all)
            nc.sync.dma_start(out=dbg_att_d.ap(), in_=att_f32)

        # ---- output projection + residual ----------------------------------
        opool = ctx.enter_context(tc.tile_pool(name="opool", bufs=2))
        for i in range(4):
            for tci in range(2):
                tsl = slice(512 * tci, 512 * (tci + 1))
                pr_ps = psum.tile([128, 512], F32, tag="qk", bufs=3,
                                  name=f"pr_{i}_{tci}")
                for kc in range(4):
                    nc.tensor.matmul(
                        pr_ps, wp[:, kc, 128 * i:128 * (i + 1)],
                        att_all[:, kc, tsl],
                        start=(kc == 0), stop=(kc == 3),
                    )
                out_sb = opool.tile([128, 512], F32, tag="osb")
                nc.vector.scalar_tensor_tensor(
                    out=out_sb, in0=pr_ps, scalar=pb[:, i:i + 1],
                    in1=x_sb[:, i, tsl], op0=ALU.add, op1=ALU.add,
                )
                nc.sync.dma_start(out=out_d.ap()[:, i, tsl], in_=out_sb)

    nc.compile()
    return nc


def _to_part_major(a, inner):
    """[C, inner...] with C=512 -> [128, 4, inner] (c = 128*i + p)."""
    return np.ascontiguousarray(
        a.reshape(4, 128, inner).transpose(1, 0, 2))


def prep_inputs(x, encoder_out, capt_attn_mask, norm_scale, norm_bias,
                qkv_w, qkv_b, ekv_w, ekv_b, proj_w, proj_b):
    """Host-side marshalling: shard over batch + transpose/cast weights."""
    bf16 = ml_dtypes.bfloat16
    x = np.asarray(x, np.float32).reshape(B, C, T)
    enc = np.asarray(encoder_out, np.float32)
    mask = np.asarray(capt_attn_mask).astype(bool)

    x_dev = x.reshape(B, 4, 128, T).transpose(0, 2, 1, 3)
    enc_dev = enc.reshape(B, 4, 128, S).transpose(0, 2, 1, 3).astype(bf16)
    addm = np.zeros((B, 128, 1), np.float32)
    addm[:, :S, 0] = np.where(mask, 0.0, MASK_NEG)
    addm_dve = B16 + A16 * addm

    # weight rows permuted into per-pair block layout:
    # [q_h | q_h1 | k_h | k_h1 | v_h | v_h1] so each matmul lhsT is one
    # contiguous 128-column slice.
    qperm = np.array([
        192 * (2 * p + hh) + 64 * b + o
        for p in range(4) for b in range(3) for hh in range(2) for o in range(64)
    ])
    eperm = np.array([
        128 * (2 * p + hh) + 64 * b + o
        for p in range(4) for b in range(2) for hh in range(2) for o in range(64)
    ])
    wq_t = _to_part_major(np.asarray(qkv_w, np.float32)[qperm].T, 3 * C).astype(bf16)
    we_t = _to_part_major(np.asarray(ekv_w, np.float32)[eperm].T, 2 * C).astype(bf16)
    wp_t = _to_part_major(np.asarray(proj_w, np.float32).T, C).astype(bf16)

    qkv_b = np.asarray(qkv_b, np.float32)
    ekv_b = np.asarray(ekv_b, np.float32)
    qb = np.zeros((128, 4, 3), np.float32)
    ebb = np.zeros((128, 4, 2), np.float32)
    for p in range(4):
        h = 2 * p
        for bi in range(3):
            qb[0:64, p, bi] = qkv_b[192 * h + 64 * bi: 192 * h + 64 * bi + 64]
            qb[64:128, p, bi] = qkv_b[192 * (h + 1) + 64 * bi: 192 * (h + 1) + 64 * bi + 64]
        for bi in range(2):
            ebb[0:64, p, bi] = ekv_b[128 * h + 64 * bi: 128 * h + 64 * bi + 64]
            ebb[64:128, p, bi] = ekv_b[128 * (h + 1) + 64 * bi: 128 * (h + 1) + 64 * bi + 64]
    pbm = np.ascontiguousarray(np.asarray(proj_b, np.float32).reshape(4, 128).T)
    gamm = np.ascontiguousarray(np.asarray(norm_scale, np.float32).reshape(4, 128).T)
    betm = np.ascontiguousarray(np.asarray(norm_bias, np.float32).reshape(4, 128).T)

    shared = {"wq": wq_t, "we": we_t, "wp": wp_t, "qb": qb, "eb": ebb,
              "pb": pbm, "gam": gamm, "bet": betm}
    in_maps = []
    for b in range(B):
        m = dict(shared)
        m["x"] = np.ascontiguousarray(x_dev[b])
        m["enc"] = np.ascontiguousarray(enc_dev[b])
        m["addma"] = np.ascontiguousarray(addm[b])
        m["addmd"] = np.ascontiguousarray(addm_dve[b])
        in_maps.append(m)
    return in_maps


def gather_output(results):
    out = np.stack([r["out"] for r in results])  # [8, 128, 4, T]
    return np.ascontiguousarray(
        out.transpose(0, 2, 1, 3).reshape(B, C, HH, WW).astype(np.float32))


_NC = None


def _get_nc():
    global _NC
    if _NC is None:
        _NC = build_program()
    return _NC


def kernel(**inputs) -> np.ndarray:
    from concourse.bass_utils import run_bass_kernel_spmd

    nc = _get_nc()
    in_maps = prep_inputs(**inputs)
    res = run_bass_kernel_spmd(nc, in_maps, core_ids=list(range(N_CORES)))
    return gather_output(res.results)


if __name__ == "__main__":
    nc = build_program()
    print("program built ok")

